# revision 1
# baseline (speedup 1.0000x reference)
"""DeepFactor (K relu-LSTM branches + shared Dense head) on 8 trn2 NeuronCores.

Sharding: the K=10 factor branches are expert-split across cores, 2 slots
per core (16 slots = 10 real + 6 zero-padded; zero weights keep the padded
slot's state identically 0 so padding is exact). Every core runs the same
SPMD program over the full batch B=32.

On-chip layout: recurrent state h/c live as [128, B] SBUF tiles
(partitions = 64 hidden units x 2 k-slots, free dim = batch). Each step,
with gate g ranging over f | i,o,c (f in its own PSUM bank so the f-path
starts early):
  - matmul  z_g  = [W_g|b_g].T @ [x_t;1]     (start=True,  contract 33)
  - matmul  z_g += blockdiag(U_k0,U_k1).T @ h (start=False, contract 128)
  - sigmoid(z_f) -> sf, then sigmoid(z_io) -> sio
  - DVE: t2=sf*c, t1=relu(zc)*si, c=t1+t2, h=relu(c)*so
    (relu(zc)*i == i*relu(zc) and relu(c)*o == o*relu(c) since i,o>0)
  - matmul  y_t = h.T @ [Wd;Wd]  -> one PSUM column (sums both slots)
Host gathers: y = (sum over cores of Y)/K + bd.
"""

import os
from contextlib import ExitStack

import numpy as np

import concourse.bass as bass
import concourse.tile as tile
from concourse import bacc, mybir
from concourse.bass_utils import run_bass_kernel_spmd

# Problem dims (hardcoded per contract)
B, T, D, U, K = 32, 1024, 32, 64, 10
NCORES = 8
CHUNK_STEPS = int(os.environ.get("KERNEL_CHUNK_STEPS", "128"))  # x timesteps per SBUF chunk

FP16 = os.environ.get("KERNEL_FP16", "1") == "1"
# scheduling variant knobs (tuned via TimelineSim cost model)
Y_MODE = os.environ.get("KERNEL_Y_MODE", "first")  # defer | first | none
SINGLE_Z = os.environ.get("KERNEL_SINGLE_Z", "0") == "1"
T2_ENGINE = os.environ.get("KERNEL_T2_ENGINE", "vector")  # vector | gpsimd
# split the two k-slots into independent [64,B] chains that interleave
SPLIT_SLOTS = os.environ.get("KERNEL_SPLIT_SLOTS", "0") == "1"
# v3 body: x-matmuls a step ahead, single sigmoid, relu(zc) on DVE in parallel
V3 = os.environ.get("KERNEL_V3", "0") == "1"
# double-buffer the h state so the DVE h-update never WAR-waits on PE readers
H_DB = os.environ.get("KERNEL_H_DB", "0") == "1"
# emit sf right after the f-pair (narrow its semaphore wait) and keep t1/t2
# as persistent all-DVE tiles (no pool-slot sems on the DVE seq)
TIGHT = os.environ.get("KERNEL_TIGHT", "0") == "1"
# precompute relu(zc) on DVE during the sigmoid window so t1 becomes a
# fast SBUF-only multiply instead of a PSUM-operand scalar_tensor_tensor
RZC = os.environ.get("KERNEL_RZC", "0") == "1"
# run the io-sigmoid (which gates the critical t1) before the f-sigmoid
SIO_FIRST = os.environ.get("KERNEL_SIO_FIRST", "0") == "1"
# 3-way sigmoid split: sf, si, so as separate ACT instrs (si before so)
SIG3 = os.environ.get("KERNEL_SIG3", "0") == "1"
# wrap the 4-op DVE block in tc.tile_critical() to merge its sem waits
CRIT = os.environ.get("KERNEL_CRIT", "0") == "1"
# sigmoid outputs in fp16 (narrower DVE reads on the chain ops)
SIG16 = os.environ.get("KERNEL_SIG16", "0") == "1"

# gate order in the reference weights (Keras): i|f|c|o
_REF_GATE_SLICE = {"i": 0, "f": 1, "c": 2, "o": 3}
# our gate order: f alone (bank 0), then i|o|c (bank 1)
_OUR_GATES = ["f", "i", "o", "c"]


def _np_dt():
    return np.float16 if FP16 else np.float32


def _mm_dt():
    return mybir.dt.float16 if FP16 else mybir.dt.float32


def _build_core_inputs(x, W, U_rec, b, Wd):
    """Per-core numpy input dicts. Slot assignment: core0:(k0,k1), core1:(k2,k3),
    cores 2-7: (k4+i, pad)."""
    ndt = _np_dt()
    xt = np.ascontiguousarray(np.transpose(x, (2, 1, 0)).reshape(D, T * B))
    xaug = np.concatenate([xt, np.ones((1, T * B), np.float32)], axis=0).astype(ndt)

    slot_ks = [(0, 1), (2, 3)] + [(4 + i, None) for i in range(6)]

    in_maps = []
    for core in range(NCORES):
        ks = slot_ks[core]
        LX = np.zeros((4, D + 1, 2 * U), np.float32)  # [gate, 33, 128]
        LH = np.zeros((4, 2 * U, 2 * U), np.float32)  # [gate, 128, 128] blockdiag
        WD2 = np.zeros((2 * U, 1), np.float32)
        for s, k in enumerate(ks):
            if k is None:
                continue
            for g, gname in enumerate(_OUR_GATES):
                ref_g = _REF_GATE_SLICE[gname]
                cols = slice(ref_g * U, (ref_g + 1) * U)
                LX[g, :D, s * U:(s + 1) * U] = W[k][:, cols]
                LX[g, D, s * U:(s + 1) * U] = b[k][cols]
                LH[g, s * U:(s + 1) * U, s * U:(s + 1) * U] = U_rec[k][:, cols]
            WD2[s * U:(s + 1) * U, 0] = Wd[:, 0]
        in_maps.append(
            {
                "xaug": xaug,
                "lx": np.ascontiguousarray(LX.astype(ndt)),
                "lh": np.ascontiguousarray(LH.astype(ndt)),
                "wd2": WD2.astype(ndt),
            }
        )
    return in_maps


def _build_program(t_steps: int) -> bacc.Bacc:
    nc = bacc.Bacc(
        "TRN2",
        target_bir_lowering=False,
        debug=False,
        enable_asserts=False,
        num_devices=NCORES,
    )
    MDT = _mm_dt()
    F32 = mybir.dt.float32
    xaug_ap = nc.dram_tensor("xaug", [D + 1, T * B], MDT, kind="ExternalInput").ap()
    lx_ap = nc.dram_tensor("lx", [4, D + 1, 2 * U], MDT, kind="ExternalInput").ap()
    lh_ap = nc.dram_tensor("lh", [4, 2 * U, 2 * U], MDT, kind="ExternalInput").ap()
    wd2_ap = nc.dram_tensor("wd2", [2 * U, 1], MDT, kind="ExternalInput").ap()
    y_ap = nc.dram_tensor("y", [B, t_steps], F32, kind="ExternalOutput").ap()

    P = 2 * U  # 128
    n_ybanks = (t_steps + 511) // 512
    sig_f = mybir.ActivationFunctionType.Sigmoid
    mmax = mybir.AluOpType.max
    mmult = mybir.AluOpType.mult

    with tile.TileContext(nc) as tc, ExitStack() as ctx:
        const_pool = ctx.enter_context(tc.tile_pool(name="const", bufs=1))
        state_pool = ctx.enter_context(tc.tile_pool(name="state", bufs=1))
        xch_pool = ctx.enter_context(tc.tile_pool(name="xch", bufs=2))
        zf_pool = ctx.enter_context(tc.tile_pool(name="zf", bufs=int(os.environ.get("KERNEL_ZF_BUFS", "2")), space="PSUM"))
        z_pool = ctx.enter_context(tc.tile_pool(name="z", bufs=int(os.environ.get("KERNEL_Z_BUFS", "3")), space="PSUM"))
        ypsum_pool = ctx.enter_context(tc.tile_pool(name="yps", bufs=1, space="PSUM"))
        s_pool = ctx.enter_context(tc.tile_pool(name="sig", bufs=int(os.environ.get("KERNEL_S_BUFS", "3"))))
        t_pool = ctx.enter_context(tc.tile_pool(name="tmp", bufs=int(os.environ.get("KERNEL_T_BUFS", "3"))))
        out_pool = ctx.enter_context(tc.tile_pool(name="out", bufs=1))

        # --- static weights into SBUF ---
        lx_tiles = []
        lh_tiles = []
        for g in range(4):
            lxg = const_pool.tile([D + 1, P], MDT, tag=f"lx{g}", name=f"lxt{g}")
            nc.sync.dma_start(lxg[:], lx_ap[g])
            lx_tiles.append(lxg)
            lhg = const_pool.tile([P, P], MDT, tag=f"lh{g}", name=f"lht{g}")
            nc.sync.dma_start(lhg[:], lh_ap[g])
            lh_tiles.append(lhg)
        wd2 = const_pool.tile([P, 1], MDT, tag="wd2")
        nc.sync.dma_start(wd2[:], wd2_ap[:])

        # --- persistent state ---
        h2 = state_pool.tile([P, B], MDT, tag="h2")
        h2b = state_pool.tile([P, B], MDT, tag="h2b")
        c2 = state_pool.tile([P, B], F32, tag="c2")
        nc.vector.memset(h2[:], 0.0)
        nc.vector.memset(h2b[:], 0.0)
        nc.vector.memset(c2[:], 0.0)
        htiles = [h2, h2b]
        t1p = state_pool.tile([P, B], F32, tag="t1p")
        t2p = state_pool.tile([P, B], F32, tag="t2p")

        ypsums = []
        for i in range(n_ybanks):
            yp = ypsum_pool.tile([B, 512], F32, tag=f"yp{i}", name=f"ypt{i}")
            ypsums.append(yp)

        def h_read(t):
            return htiles[(t + 1) % 2] if H_DB else h2

        def h_write(t):
            return htiles[t % 2] if H_DB else h2

        def mm_pair(out_ap, g, xrhs, hprev):
            nc.tensor.matmul(
                out_ap, lhsT=lx_tiles[g][:], rhs=xrhs,
                start=True, stop=False, skip_group_check=True,
            )
            nc.tensor.matmul(
                out_ap, lhsT=lh_tiles[g][:], rhs=hprev[:],
                start=False, stop=True, skip_group_check=True,
            )

        def y_mm(t):
            if Y_MODE == "none":
                return
            nc.tensor.matmul(
                ypsums[t // 512][:, (t % 512):(t % 512) + 1],
                lhsT=h_write(t)[:], rhs=wd2[:], start=True, stop=True,
            )

        t2_eng = nc.gpsimd if T2_ENGINE == "gpsimd" else nc.vector

        if SPLIT_SLOTS:
            zs_pool = ctx.enter_context(
                tc.tile_pool(name="zs", bufs=2, space="PSUM")
            )
            # per-slot weight tiles at base partition 0
            lxs = [[None, None] for _ in range(4)]
            lhs = [[None, None] for _ in range(4)]
            wds = [None, None]
            for s in range(2):
                su = s * U
                for g in range(4):
                    lxg = const_pool.tile(
                        [D + 1, U], MDT, tag=f"lxs{g}_{s}", name=f"lxs{g}_{s}"
                    )
                    nc.sync.dma_start(lxg[:], lx_ap[g][:, su:su + U])
                    lxs[g][s] = lxg
                    lhg = const_pool.tile(
                        [U, U], MDT, tag=f"lhs{g}_{s}", name=f"lhs{g}_{s}"
                    )
                    nc.sync.dma_start(lhg[:], lh_ap[g][su:su + U, su:su + U])
                    lhs[g][s] = lhg
                wdt = const_pool.tile([U, 1], MDT, tag=f"wds{s}", name=f"wds{s}")
                nc.sync.dma_start(wdt[:], wd2_ap[su:su + U])
                wds[s] = wdt
            hs = []
            cs = []
            for s in range(2):
                hsx = state_pool.tile([U, B], MDT, tag=f"hs{s}", name=f"hs{s}")
                csx = state_pool.tile([U, B], F32, tag=f"cs{s}", name=f"cs{s}")
                nc.vector.memset(hsx[:], 0.0)
                nc.vector.memset(csx[:], 0.0)
                hs.append(hsx)
                cs.append(csx)

            xch = None
            for t in range(t_steps):
                if t % CHUNK_STEPS == 0:
                    n_cols = min(CHUNK_STEPS, t_steps - t) * B
                    xch = xch_pool.tile([D + 1, CHUNK_STEPS * B], MDT, tag="xch")
                    nc.sync.dma_start(
                        xch[:, 0:n_cols], xaug_ap[:, t * B:t * B + n_cols]
                    )
                off = (t % CHUNK_STEPS) * B
                xrhs = xch[:, off:off + B]

                zslots = []
                for s in range(2):
                    su = s * U
                    z = zs_pool.tile([U, 4 * B], F32, tag=f"z{s}", name=f"z{s}_{t}")
                    for g in range(4):
                        nc.tensor.matmul(
                            z[:, g * B:(g + 1) * B],
                            lhsT=lxs[g][s][:],
                            rhs=xrhs,
                            start=True, stop=False, skip_group_check=True,
                        )
                        nc.tensor.matmul(
                            z[:, g * B:(g + 1) * B],
                            lhsT=lhs[g][s][:],
                            rhs=hs[s][:],
                            start=False, stop=True, skip_group_check=True,
                        )
                    zslots.append(z)

                if t > 0 and Y_MODE != "none":
                    tp = t - 1
                    yap = ypsums[tp // 512][:, (tp % 512):(tp % 512) + 1]
                    nc.tensor.matmul(
                        yap, lhsT=hs[0][:], rhs=wds[0][:], start=True, stop=False,
                    )
                    nc.tensor.matmul(
                        yap, lhsT=hs[1][:], rhs=wds[1][:], start=False, stop=True,
                    )

                for s in range(2):
                    z = zslots[s]
                    sig = s_pool.tile([U, 3 * B], F32, tag=f"sig{s}", name=f"sg{s}_{t}")
                    nc.scalar.activation(sig[:], z[:, 0:3 * B], sig_f)
                    sf, si, so = sig[:, 0:B], sig[:, B:2 * B], sig[:, 2 * B:3 * B]
                    zc = z[:, 3 * B:4 * B]
                    t2 = t_pool.tile([U, B], F32, tag=f"t2{s}", name=f"t2{s}_{t}")
                    t2_eng.tensor_mul(t2[:], sf, cs[s][:])
                    t1 = t_pool.tile([U, B], F32, tag=f"t1{s}", name=f"t1{s}_{t}")
                    nc.vector.scalar_tensor_tensor(
                        t1[:], zc, 0.0, si, op0=mmax, op1=mmult
                    )
                    nc.vector.tensor_add(cs[s][:], t1[:], t2[:])
                    nc.vector.scalar_tensor_tensor(
                        hs[s][:], cs[s][:], 0.0, so, op0=mmax, op1=mmult
                    )

            if Y_MODE != "none":
                tp = t_steps - 1
                yap = ypsums[tp // 512][:, (tp % 512):(tp % 512) + 1]
                nc.tensor.matmul(
                    yap, lhsT=hs[0][:], rhs=wds[0][:], start=True, stop=False,
                )
                nc.tensor.matmul(
                    yap, lhsT=hs[1][:], rhs=wds[1][:], start=False, stop=True,
                )

        if V3 and not SPLIT_SLOTS:
            # x-projections land in z(t+1) during step t; critical window per
            # step is 4 recurrent matmuls -> 1 sigmoid -> 4 DVE ops.
            xch = None

            def load_chunk(t):
                n_cols = min(CHUNK_STEPS, t_steps - t) * B
                xc = xch_pool.tile([D + 1, CHUNK_STEPS * B], MDT, tag="xch")
                nc.sync.dma_start(
                    xc[:, 0:n_cols], xaug_ap[:, t * B:t * B + n_cols]
                )
                return xc

            def emit_x_mms(t, xc):
                z = z_pool.tile([P, 4 * B], F32, tag="z", name=f"z_{t}")
                off = (t % CHUNK_STEPS) * B
                for g in range(4):
                    nc.tensor.matmul(
                        z[:, g * B:(g + 1) * B],
                        lhsT=lx_tiles[g][:], rhs=xc[:, off:off + B],
                        start=True, stop=False, skip_group_check=True,
                    )
                return z

            xch = load_chunk(0)
            z_cur = emit_x_mms(0, xch)
            for t in range(t_steps):
                for g in range(4):
                    nc.tensor.matmul(
                        z_cur[:, g * B:(g + 1) * B],
                        lhsT=lh_tiles[g][:], rhs=h2[:],
                        start=False, stop=True, skip_group_check=True,
                    )
                if t > 0 and Y_MODE != "none":
                    y_mm(t - 1)
                if t + 1 < t_steps:
                    if (t + 1) % CHUNK_STEPS == 0:
                        xch = load_chunk(t + 1)
                    z_next = emit_x_mms(t + 1, xch)

                rzc = t_pool.tile([P, B], F32, tag="rzc", name=f"rzc_{t}")
                nc.vector.tensor_scalar_max(rzc[:], z_cur[:, 3 * B:4 * B], 0.0)
                sig = s_pool.tile([P, 3 * B], F32, tag="sig", name=f"sg_{t}")
                nc.scalar.activation(sig[:], z_cur[:, 0:3 * B], sig_f)

                t2 = t_pool.tile([P, B], F32, tag="t2", name=f"t2_{t}")
                t2_eng.tensor_mul(t2[:], sig[:, 0:B], c2[:])
                t1 = t_pool.tile([P, B], F32, tag="t1", name=f"t1_{t}")
                nc.vector.tensor_mul(t1[:], sig[:, B:2 * B], rzc[:])
                nc.vector.tensor_add(c2[:], t1[:], t2[:])
                nc.vector.scalar_tensor_tensor(
                    h2[:], c2[:], 0.0, sig[:, 2 * B:3 * B], op0=mmax, op1=mmult
                )
                if t + 1 < t_steps:
                    z_cur = z_next
            if Y_MODE != "none":
                y_mm(t_steps - 1)

        if not SPLIT_SLOTS and not V3:
          xch = None
          prev_h_mm = None  # deferred y-projection emission
          for t in range(t_steps):
            if t % CHUNK_STEPS == 0:
                n_cols = min(CHUNK_STEPS, t_steps - t) * B
                xch = xch_pool.tile([D + 1, CHUNK_STEPS * B], MDT, tag="xch")
                nc.sync.dma_start(
                    xch[:, 0:n_cols], xaug_ap[:, t * B:t * B + n_cols]
                )
            off = (t % CHUNK_STEPS) * B
            xrhs = xch[:, off:off + B]

            if Y_MODE == "first" and t > 0:
                y_mm(t - 1)

            hprev = h_read(t)
            if SINGLE_Z:
                zall = z_pool.tile([P, 4 * B], F32, tag="zioc")
                zf = zall[:, 0:B]
                zioc = zall[:, B:4 * B]
                mm_pair(zf, 0, xrhs, hprev)
                for g in (1, 2, 3):
                    mm_pair(zall[:, g * B:(g + 1) * B], g, xrhs, hprev)
            else:
                zf_t = zf_pool.tile([P, B], F32, tag="zf")
                zf = zf_t[:]
                zioc = z_pool.tile([P, 3 * B], F32, tag="zioc")
                mm_pair(zf, 0, xrhs, hprev)
                if TIGHT:
                    sf_t = s_pool.tile([P, B], F32, tag="sf")
                    nc.scalar.activation(sf_t[:], zf, sig_f)
                    sf = sf_t[:]
                for g in (1, 2, 3):  # i, o, c
                    mm_pair(zioc[:, (g - 1) * B:g * B], g, xrhs, hprev)

            if Y_MODE == "defer" and prev_h_mm is not None:
                y_mm(prev_h_mm)
            prev_h_mm = t

            if SINGLE_Z:
                sig = s_pool.tile([P, 3 * B], F32, tag="sig")
                nc.scalar.activation(sig[:], zall[:, 0:3 * B], sig_f)
                sf, si, so = sig[:, 0:B], sig[:, B:2 * B], sig[:, 2 * B:3 * B]
                zc = zall[:, 3 * B:4 * B]
            elif SIG3:
                sf_t = s_pool.tile([P, B], F32, tag="sf")
                nc.scalar.activation(sf_t[:], zf, sig_f)
                sf = sf_t[:]
                si_t = s_pool.tile([P, B], F32, tag="si3")
                nc.scalar.activation(si_t[:], zioc[:, 0:B], sig_f)
                so_t = s_pool.tile([P, B], F32, tag="so3")
                nc.scalar.activation(so_t[:], zioc[:, B:2 * B], sig_f)
                si, so = si_t[:], so_t[:]
                zc = zioc[:, 2 * B:3 * B]
            elif SIO_FIRST:
                sio = s_pool.tile([P, 2 * B], F32, tag="sio")
                nc.scalar.activation(sio[:], zioc[:, 0:2 * B], sig_f)
                sf_t = s_pool.tile([P, B], F32, tag="sf")
                nc.scalar.activation(sf_t[:], zf, sig_f)
                sf = sf_t[:]
                si, so = sio[:, 0:B], sio[:, B:2 * B]
                zc = zioc[:, 2 * B:3 * B]
            else:
                SDT = mybir.dt.float16 if SIG16 else F32
                if not TIGHT:
                    sf_t = s_pool.tile([P, B], SDT, tag="sf")
                    nc.scalar.activation(sf_t[:], zf, sig_f)
                    sf = sf_t[:]
                sio = s_pool.tile([P, 2 * B], SDT, tag="sio")
                nc.scalar.activation(sio[:], zioc[:, 0:2 * B], sig_f)
                si, so = sio[:, 0:B], sio[:, B:2 * B]
                zc = zioc[:, 2 * B:3 * B]

            if TIGHT:
                t2, t1 = t2p, t1p
            else:
                t2 = t_pool.tile([P, B], F32, tag="t2")
                t1 = t_pool.tile([P, B], F32, tag="t1")
            if RZC:
                rzc = t_pool.tile([P, B], F32, tag="rzc")
                nc.vector.tensor_scalar_max(rzc[:], zc, 0.0)
            if CRIT:
                from contextlib import nullcontext
                crit_ctx = tc.tile_critical()
            else:
                from contextlib import nullcontext
                crit_ctx = nullcontext()
            with crit_ctx:
                if SIO_FIRST:
                    nc.vector.scalar_tensor_tensor(
                        t1[:], zc, 0.0, si, op0=mmax, op1=mmult
                    )
                    t2_eng.tensor_mul(t2[:], sf, c2[:])
                else:
                    t2_eng.tensor_mul(t2[:], sf, c2[:])
                    # t1 = relu(z_c) * sig_i
                    if RZC:
                        nc.vector.tensor_mul(t1[:], rzc[:], si)
                    else:
                        nc.vector.scalar_tensor_tensor(
                            t1[:], zc, 0.0, si, op0=mmax, op1=mmult
                        )
                nc.vector.tensor_add(c2[:], t1[:], t2[:])
                # h = relu(c) * sig_o
                nc.vector.scalar_tensor_tensor(
                    h_write(t)[:], c2[:], 0.0, so, op0=mmax, op1=mmult
                )

          if Y_MODE != "none":
            tp = prev_h_mm
            nc.tensor.matmul(
                ypsums[tp // 512][:, (tp % 512):(tp % 512) + 1],
                lhsT=h_write(tp)[:], rhs=wd2[:], start=True, stop=True,
            )

        ysb = out_pool.tile([B, t_steps], F32, tag="ysb")
        for i in range(n_ybanks):
            n = min(512, t_steps - i * 512)
            nc.scalar.copy(ysb[:, i * 512:i * 512 + n], ypsums[i][:, 0:n])
        nc.sync.dma_start(y_ap[:, :], ysb[:])

    nc.compile()
    return nc


def kernel(x, W, U_rec, b, Wd, bd):
    x = np.asarray(x, np.float32)
    W = np.asarray(W, np.float32)
    U_rec = np.asarray(U_rec, np.float32)
    b = np.asarray(b, np.float32)
    Wd = np.asarray(Wd, np.float32)
    bd = np.asarray(bd, np.float32)

    in_maps = _build_core_inputs(x, W, U_rec, b, Wd)
    nc = _build_program(T)
    res = run_bass_kernel_spmd(nc, in_maps, core_ids=list(range(NCORES)))
    ysum = np.zeros((B, T), np.float64)
    for r in res.results:
        ysum += r["y"].astype(np.float64)
    y = (ysum / K + bd[0]).astype(np.float32)
    return y[:, :, None]


if __name__ == "__main__":
    rng = np.random.default_rng(0)
    out = kernel(
        rng.standard_normal((B, T, D), np.float32),
        rng.standard_normal((K, D, 4 * U), np.float32) * 0.05,
        rng.standard_normal((K, U, 4 * U), np.float32) * 0.05,
        np.zeros((K, 4 * U), np.float32),
        rng.standard_normal((U, 1), np.float32) * 0.05,
        np.zeros((1,), np.float32),
    )
    print(out.shape, out.dtype)



# revision 15
# speedup vs baseline: 1.1777x; 1.1777x over previous
"""DeepFactor (K relu-LSTM branches + shared Dense head) on 8 trn2 NeuronCores.

Sharding: the K=10 factor branches are expert-split across cores, 2 slots
per core (16 slots = 10 real + 6 zero-padded; zero weights keep the padded
slot's state identically 0 so padding is exact). Every core runs the same
SPMD program over the full batch B=32.

The per-step recurrence is latency-bound on the PE->ACT->DVE->PE loop when
sigmoids run on the scalar engine. This kernel removes ACT from the loop
entirely with an exp-free sigmoid evaluated on the DVE via custom ops:

    sigma(z) ~= recip_1NR(1 + (1 - z/beta)^256)

where (1-z/beta)^64 is 6 squarings (POW64 op, one instruction for the
f|i|o gate block) and recip_1NR is the BITWISE_NOT exponent-flip seed plus
one Newton step (fused with the gate multiply in SIGR_MUL / SIGR_MUL_RELU).
Max |sigma_hat - sigma| ~= 3e-3, near-zero mean (validated end-to-end in
numpy against the jax reference before pinning the constants).

Per step, gate columns f|i|o|c in one PSUM tile:
  PE :  z_g  = [W_g|b_g].T @ [x_t;1]  (start)  ... x-projection, prefetched
        z_g += blockdiag(U_k0,U_k1).T @ h (stop)  4 recurrent matmuls
  DVE:  v  = (1 - z[f|i|o]/beta)^64                   POW64
        t2 = sigr(v_f) * c           = sigma(zf)*c    SIGR_MUL
        t1 = sigr(v_i) * relu(zc)    = sigma(zi)*relu(zc)  SIGR_MUL_RELU
        c' = t1 + t2
        h  = sigr(v_o) * relu(c')    = sigma(zo)*relu(c')  SIGR_MUL_RELU
  PE :  y_t = h.T @ [Wd;Wd]   (one PSUM column; sums both k slots)
Host gathers: y = (sum over cores of Y)/K + bd.
"""

import os
from contextlib import ExitStack

import numpy as np

import concourse.bass as bass
import concourse.tile as tile
from concourse import bacc, mybir
from concourse.bass_utils import run_bass_kernel_spmd

# Problem dims (hardcoded per contract)
B, T, D, U, K = 32, 1024, 32, 64, 10
NCORES = 8
CHUNK_STEPS = int(os.environ.get("KERNEL_CHUNK_STEPS", "128"))

FP16 = os.environ.get("KERNEL_FP16", "1") == "1"
H_DB = os.environ.get("KERNEL_H_DB", "1") == "1"  # double-buffer h state
Z_BUFS = int(os.environ.get("KERNEL_Z_BUFS", "3"))
V_BUFS = int(os.environ.get("KERNEL_V_BUFS", "2"))
Y_MM = os.environ.get("KERNEL_Y_MM", "1") == "1"

# sigma_hat constants (fit vs true sigmoid; see module docstring). The
# Newton constant TNR absorbs the NR undershoot correction (T = 2+delta).
# (1 - z/beta) is folded into the f|i|o weights, so the matmul emits w
# directly and the POW op is 8 pure squarings (n=256).
SIG_BETA = 256.7562003289679
SIG_C0SEED = -0.23594391924053412
SIG_TNR = 2.00162127342384
D_AUG = D + 2  # x rows + exact-1.0 row + (-b/beta) row

# gate order in the reference weights (Keras): i|f|c|o
_REF_GATE_SLICE = {"i": 0, "f": 1, "c": 2, "o": 3}
# our gate order: f|i|o (sigma_hat block) then c (relu'd on DVE)
_OUR_GATES = ["f", "i", "o", "c"]


# --- custom DVE ops (registered into concourse.dve_ops at import) -----------
def _register_custom_ops():
    from concourse import dve_ops
    from concourse.dve_spec import (
        Spec, Src0, Src1, C0, C1, One, Zero, AluOp, Bin, lower, maxx,
        _has_src1,
    )
    from concourse.dve_uop import DveOpSpec

    if "DF_POW256_ANT" in dve_ops._SUB_OPCODE_FOR_NAME:
        return  # already registered in this process

    def _pow256_ref(in0, in1, s0, s1, imm2):
        v = in0.astype(np.float32)
        for _ in range(8):
            v = (v * v).astype(np.float32)
        return v

    v = Src0
    for _ in range(8):
        v = v * v
    pow256_spec = Spec(body=v, reference=_pow256_ref)

    def _sig_core(src1_term):
        # sigma_hat(z)*x = y0*(C1 - d*y0) * x, d = 1+v, seed y0 = ~bits(d)*C0
        d = One + Src0
        nb = Bin(AluOp.BITWISE_NOT, d, d)
        y0 = nb * C0
        y1 = y0 * (C1 - d * y0)
        return y1 * src1_term

    def _sigr_ref(relu):
        def ref(in0, in1, s0, s1, imm2):
            d = (1.0 + in0).astype(np.float32)
            nb = (~d.view(np.int32)).view(np.float32)
            y0 = (nb * np.float32(s0)).astype(np.float32)
            y1 = (y0 * (np.float32(s1) - d * y0)).astype(np.float32)
            t = np.maximum(in1, 0) if relu else in1
            return (y1 * t).astype(np.float32)
        return ref

    sigr_mul_spec = Spec(body=_sig_core(Src1), reference=_sigr_ref(False))
    sigr_mul_relu_spec = Spec(
        body=_sig_core(maxx(Src1, Zero)), reference=_sigr_ref(True)
    )

    ops = []
    for name, spec in (
        ("DF_POW256_ANT", pow256_spec),
        ("DF_SIGR_MUL_ANT", sigr_mul_spec),
        ("DF_SIGR_MUL_RELU_ANT", sigr_mul_relu_spec),
    ):
        row = dve_ops._CUSTOM_DVE_ROW_BASE + len(dve_ops.OPS)
        shas = {}
        for ver in ("v3", "v4"):
            uops = lower(spec, ver=ver)
            s = DveOpSpec(
                name=name, opcode=row, uops=uops, rd1_en=_has_src1(spec)
            )
            shas[ver] = s.sha(ver)
        op = dve_ops.DveOp(name, spec, subdim=False, uops_sha=shas)
        dve_ops.OPS.append(op)
        dve_ops.CUSTOM_DVE_SPECS[name] = spec
        dve_ops._SUB_OPCODE_FOR_NAME[name] = row
        ops.append(op)
    return ops


_register_custom_ops()


def _np_dt():
    return np.float16 if FP16 else np.float32


def _mm_dt():
    return mybir.dt.float16 if FP16 else mybir.dt.float32


def _build_core_inputs(x, W, U_rec, b, Wd):
    """Per-core numpy input dicts. Slot assignment: core0:(k0,k1), core1:(k2,k3),
    cores 2-7: (k4+i, pad)."""
    ndt = _np_dt()
    xt = np.ascontiguousarray(np.transpose(x, (2, 1, 0)).reshape(D, T * B))
    xaug = np.concatenate(
        [xt, np.ones((2, T * B), np.float32)], axis=0
    ).astype(ndt)

    slot_ks = [(0, 1), (2, 3)] + [(4 + i, None) for i in range(6)]
    neg_sc = -1.0 / SIG_BETA

    in_maps = []
    for core in range(NCORES):
        ks = slot_ks[core]
        LX = np.zeros((4, D_AUG, 2 * U), np.float32)  # [gate, 34, 128]
        LH = np.zeros((4, 2 * U, 2 * U), np.float32)  # [gate, 128, 128] blockdiag
        WD2 = np.zeros((2 * U, 1), np.float32)
        for s, k in enumerate(ks):
            if k is None:
                continue
            for g, gname in enumerate(_OUR_GATES):
                ref_g = _REF_GATE_SLICE[gname]
                cols = slice(ref_g * U, (ref_g + 1) * U)
                sc = 1.0 if gname == "c" else neg_sc
                LX[g, :D, s * U:(s + 1) * U] = W[k][:, cols] * sc
                LX[g, D, s * U:(s + 1) * U] = b[k][cols] * sc
                # exact +1.0 row for w = 1 - z/beta (c gate: no offset)
                LX[g, D + 1, s * U:(s + 1) * U] = 0.0 if gname == "c" else 1.0
                LH[g, s * U:(s + 1) * U, s * U:(s + 1) * U] = (
                    U_rec[k][:, cols] * sc
                )
            WD2[s * U:(s + 1) * U, 0] = Wd[:, 0]
        in_maps.append(
            {
                "xaug": xaug,
                "lx": np.ascontiguousarray(LX.astype(ndt)),
                "lh": np.ascontiguousarray(LH.astype(ndt)),
                "wd2": WD2.astype(ndt),
            }
        )
    return in_maps


def _build_program(t_steps: int) -> bacc.Bacc:
    from concourse import dve_ops

    POW256 = next(o for o in dve_ops.OPS if o.name == "DF_POW256_ANT")
    SIGR_MUL = next(o for o in dve_ops.OPS if o.name == "DF_SIGR_MUL_ANT")
    SIGR_MUL_RELU = next(
        o for o in dve_ops.OPS if o.name == "DF_SIGR_MUL_RELU_ANT"
    )

    nc = bacc.Bacc(
        "TRN2",
        target_bir_lowering=False,
        debug=False,
        enable_asserts=False,
        num_devices=NCORES,
    )
    MDT = _mm_dt()
    F32 = mybir.dt.float32
    xaug_ap = nc.dram_tensor("xaug", [D_AUG, T * B], MDT, kind="ExternalInput").ap()
    lx_ap = nc.dram_tensor("lx", [4, D_AUG, 2 * U], MDT, kind="ExternalInput").ap()
    lh_ap = nc.dram_tensor("lh", [4, 2 * U, 2 * U], MDT, kind="ExternalInput").ap()
    wd2_ap = nc.dram_tensor("wd2", [2 * U, 1], MDT, kind="ExternalInput").ap()
    y_ap = nc.dram_tensor("y", [B, t_steps], F32, kind="ExternalOutput").ap()

    P = 2 * U  # 128
    n_ybanks = (t_steps + 511) // 512

    with tile.TileContext(nc) as tc, ExitStack() as ctx:
        const_pool = ctx.enter_context(tc.tile_pool(name="const", bufs=1))
        state_pool = ctx.enter_context(tc.tile_pool(name="state", bufs=1))
        xch_pool = ctx.enter_context(tc.tile_pool(name="xch", bufs=2))
        z_pool = ctx.enter_context(
            tc.tile_pool(name="z", bufs=Z_BUFS, space="PSUM")
        )
        ypsum_pool = ctx.enter_context(tc.tile_pool(name="yps", bufs=1, space="PSUM"))
        v_pool = ctx.enter_context(tc.tile_pool(name="v", bufs=V_BUFS))
        out_pool = ctx.enter_context(tc.tile_pool(name="out", bufs=1))

        # --- static weights into SBUF ---
        lx_tiles = []
        lh_tiles = []
        for g in range(4):
            lxg = const_pool.tile([D_AUG, P], MDT, tag=f"lx{g}", name=f"lxt{g}")
            nc.sync.dma_start(lxg[:], lx_ap[g])
            lx_tiles.append(lxg)
            lhg = const_pool.tile([P, P], MDT, tag=f"lh{g}", name=f"lht{g}")
            nc.sync.dma_start(lhg[:], lh_ap[g])
            lh_tiles.append(lhg)
        wd2 = const_pool.tile([P, 1], MDT, tag="wd2")
        nc.sync.dma_start(wd2[:], wd2_ap[:])

        # --- persistent state ---
        hs = []
        for ph in range(2 if H_DB else 1):
            t_ = state_pool.tile([P, B], MDT, tag=f"h{ph}", name=f"h{ph}")
            nc.vector.memset(t_[:], 0.0)
            hs.append(t_)
        c2 = state_pool.tile([P, B], F32, tag="c2")
        nc.vector.memset(c2[:], 0.0)
        t1p = state_pool.tile([P, B], F32, tag="t1p")
        t2p = state_pool.tile([P, B], F32, tag="t2p")

        def h_read(t):
            return hs[(t + 1) % 2] if H_DB else hs[0]

        def h_write(t):
            return hs[t % 2] if H_DB else hs[0]

        ypsums = []
        for i in range(n_ybanks):
            yp = ypsum_pool.tile([B, 512], F32, tag=f"yp{i}", name=f"ypt{i}")
            ypsums.append(yp)

        def y_mm(tp):
            nc.tensor.matmul(
                ypsums[tp // 512][:, (tp % 512):(tp % 512) + 1],
                lhsT=h_write(tp)[:], rhs=wd2[:], start=True, stop=True,
            )

        def load_chunk(t):
            n_cols = min(CHUNK_STEPS, t_steps - t) * B
            xc = xch_pool.tile([D_AUG, CHUNK_STEPS * B], MDT, tag="xch")
            nc.sync.dma_start(xc[:, 0:n_cols], xaug_ap[:, t * B:t * B + n_cols])
            return xc

        xch = load_chunk(0)
        for t in range(t_steps):
            if t % CHUNK_STEPS == 0 and t > 0:
                xch = load_chunk(t)
            off = (t % CHUNK_STEPS) * B
            hprev = h_read(t)
            # PSUM accumulation groups are bank-scoped: the start=True x-mm
            # and stop=True rec-mm of each gate must stay adjacent.
            z_cur = z_pool.tile([P, 4 * B], F32, tag="z", name=f"z_{t}")
            for g in range(4):
                zg = z_cur[:, g * B:(g + 1) * B]
                nc.tensor.matmul(
                    zg, lhsT=lx_tiles[g][:], rhs=xch[:, off:off + B],
                    start=True, stop=False, skip_group_check=True,
                )
                nc.tensor.matmul(
                    zg, lhsT=lh_tiles[g][:], rhs=hprev[:],
                    start=False, stop=True, skip_group_check=True,
                )

            if Y_MM and t > 0:
                y_mm(t - 1)

            # DVE block: v = (1 - z/beta)^64 over f|i|o, then fused gates
            v = v_pool.tile([P, 3 * B], F32, tag="v", name=f"v_{t}")
            nc.vector._custom_dve(POW256, out=v[:], in0=z_cur[:, 0:3 * B])
            # t1 first: its long PSUM-operand ack drains under t2's busy time
            nc.vector._custom_dve(
                SIGR_MUL_RELU, out=t1p[:], in0=v[:, B:2 * B],
                in1=z_cur[:, 3 * B:4 * B], s0=SIG_C0SEED, s1=SIG_TNR,
            )
            nc.vector._custom_dve(
                SIGR_MUL, out=t2p[:], in0=v[:, 0:B], in1=c2[:],
                s0=SIG_C0SEED, s1=SIG_TNR,
            )
            nc.vector.tensor_add(c2[:], t1p[:], t2p[:])
            nc.vector._custom_dve(
                SIGR_MUL_RELU, out=h_write(t)[:], in0=v[:, 2 * B:3 * B],
                in1=c2[:], s0=SIG_C0SEED, s1=SIG_TNR,
            )

        if Y_MM:
            y_mm(t_steps - 1)

        ysb = out_pool.tile([B, t_steps], F32, tag="ysb")
        for i in range(n_ybanks):
            n = min(512, t_steps - i * 512)
            nc.scalar.copy(ysb[:, i * 512:i * 512 + n], ypsums[i][:, 0:n])
        nc.sync.dma_start(y_ap[:, :], ysb[:])

    nc.compile()
    return nc


def kernel(x, W, U_rec, b, Wd, bd):
    x = np.asarray(x, np.float32)
    W = np.asarray(W, np.float32)
    U_rec = np.asarray(U_rec, np.float32)
    b = np.asarray(b, np.float32)
    Wd = np.asarray(Wd, np.float32)
    bd = np.asarray(bd, np.float32)

    in_maps = _build_core_inputs(x, W, U_rec, b, Wd)
    nc = _build_program(T)
    res = run_bass_kernel_spmd(nc, in_maps, core_ids=list(range(NCORES)))
    ysum = np.zeros((B, T), np.float64)
    for r in res.results:
        ysum += r["y"].astype(np.float64)
    y = (ysum / K + bd[0]).astype(np.float32)
    return y[:, :, None]


if __name__ == "__main__":
    rng = np.random.default_rng(0)
    out = kernel(
        rng.standard_normal((B, T, D), np.float32),
        rng.standard_normal((K, D, 4 * U), np.float32) * 0.05,
        rng.standard_normal((K, U, 4 * U), np.float32) * 0.05,
        np.zeros((K, 4 * U), np.float32),
        rng.standard_normal((U, 1), np.float32) * 0.05,
        np.zeros((1,), np.float32),
    )
    print(out.shape, out.dtype)
